# revision 18
# baseline (speedup 1.0000x reference)
"""MoE (dense all-expert routing) Trainium2 Bass kernel.

Strategy: token-parallel across 8 NeuronCores (1024 tokens each, no
collectives). Math identity used:
    out[n] = sum_e w[n,e] * (x[n] @ We[e] + be[e])
where w[n,e] = normalized top-2 softmax gate weight (0 for unselected
experts). Since softmax's denominator cancels in the top-2
renormalization, w = exp(l_e - l_max) / (exp(l_1 - l_max) + exp(l_2 - l_max))
at the top-2 logit positions, 0 elsewhere.

Per core:
  - x arrives pre-transposed (host-side layout choice): xT [D, NT]
  - gating: logits = x @ Wg + bg via PE, top-2 via DVE max8/match_replace
  - bias init: acc[t] = w^T_t @ be (small K=8 matmul)
  - main: for each expert, PSUM-accumulated [128x512] matmuls over K=D,
    then one fused DVE op: acc += w[:,e] * psum
  - store acc -> out
"""

import sys

if "/opt/trn_rl_repo" not in sys.path:
    sys.path.insert(0, "/opt/trn_rl_repo")

import numpy as np

import concourse.bass as bass
import concourse.mybir as mybir
from concourse import bacc
from concourse.bass import ds, ts
from concourse.bass_utils import run_bass_kernel_spmd
from concourse.masks import make_identity
from concourse.tile import TileContext

B, S, D, O, E = 4, 2048, 1024, 1024, 8
N = B * S            # 8192 tokens total
NCORES = 8
NT = N // NCORES     # 1024 tokens per core
P = 128
KCH = D // P         # 8 contraction chunks
TCH = NT // P        # 8 token chunks per core
OH = O // 512        # 2 output halves (512 = fp32 PSUM bank)

F32 = mybir.dt.float32
BF16 = mybir.dt.bfloat16
MM_DT = mybir.dt.float32r  # full-rate fp32 PE streaming mode


def _build():
    nc = bacc.Bacc("TRN2", target_bir_lowering=False, debug=False,
                   num_devices=NCORES)

    xT_d = nc.dram_tensor("xT", [D, NT], MM_DT, kind="ExternalInput")
    We_d = nc.dram_tensor("We", [E, D, O], MM_DT, kind="ExternalInput")
    be_d = nc.dram_tensor("be", [E, O], F32, kind="ExternalInput")
    # gating operands as bf16 hi/lo pairs: full-precision logits via
    # hi*hi + hi*lo + lo*hi (exact bf16 products, fp32 PSUM accumulate)
    xTh_d = nc.dram_tensor("xTh", [D, NT], BF16, kind="ExternalInput")
    xTl_d = nc.dram_tensor("xTl", [D, NT], BF16, kind="ExternalInput")
    Wgh_d = nc.dram_tensor("Wgh", [D, E], BF16, kind="ExternalInput")
    Wgl_d = nc.dram_tensor("Wgl", [D, E], BF16, kind="ExternalInput")
    bg_d = nc.dram_tensor("bg", [1, E], F32, kind="ExternalInput")
    out_d = nc.dram_tensor("out", [NT, O], F32, kind="ExternalOutput")

    with TileContext(nc) as tc:
        with (
            tc.tile_pool(name="const", bufs=1) as const_pool,
            tc.tile_pool(name="xT", bufs=KCH) as xT_pool,
            tc.tile_pool(name="acc", bufs=TCH) as acc_pool,
            tc.tile_pool(name="wts", bufs=16) as we_pool,
            tc.tile_pool(name="small", bufs=4) as small,
            tc.tile_pool(name="psum_mm", bufs=4, space="PSUM") as psum_mm,
            tc.tile_pool(name="psum_sm", bufs=2, space="PSUM") as psum_sm,
        ):
            # ---- constants ----
            ident = const_pool.tile([P, P], F32)
            make_identity(nc, ident)
            ones_row = const_pool.tile([1, P], F32)
            nc.vector.memset(ones_row, 1.0)
            Wgh_sb = const_pool.tile([P, KCH, E], BF16)
            nc.sync.dma_start(out=Wgh_sb, in_=Wgh_d.rearrange("(k p) e -> p k e", p=P))
            Wgl_sb = const_pool.tile([P, KCH, E], BF16)
            nc.sync.dma_start(out=Wgl_sb, in_=Wgl_d.rearrange("(k p) e -> p k e", p=P))
            bg_sb = const_pool.tile([1, E], F32)
            nc.sync.dma_start(out=bg_sb, in_=bg_d[:, :])
            be_sb = const_pool.tile([E, O], F32)
            nc.sync.dma_start(out=be_sb, in_=be_d[:, :])

            # ---- load pre-transposed activations ----
            xT = []
            xTh = []
            xTl = []
            for k in range(KCH):
                t_ = xT_pool.tile([P, NT], MM_DT, tag="xT")
                nc.sync.dma_start(out=t_, in_=xT_d[ds(k * P, P), :])
                xT.append(t_)
                th = xT_pool.tile([P, NT], BF16, tag="xTh")
                nc.sync.dma_start(out=th, in_=xTh_d[ds(k * P, P), :])
                xTh.append(th)
                tl = xT_pool.tile([P, NT], BF16, tag="xTl")
                nc.sync.dma_start(out=tl, in_=xTl_d[ds(k * P, P), :])
                xTl.append(tl)

            # ---- expert weight streaming (issued early for prefetch) ----
            wt_all = {}

            def load_expert(e):
                wt = []
                for k in range(KCH):
                    w_ = we_pool.tile([P, O], MM_DT, tag="we")
                    nc.sync.dma_start(out=w_, in_=We_d[e, ds(k * P, P), :])
                    wt.append(w_)
                wt_all[e] = wt

            load_expert(0)
            load_expert(1)

            # ---- gating: logits -> top-2 normalized weights ----
            w_all = const_pool.tile([P, TCH * E], F32)   # [token_p, t*E+e]
            wT_sb = const_pool.tile([E, NT], F32)        # transposed gates
            for t in range(TCH):
                psg = psum_sm.tile([P, E], F32, tag="psg")
                for k in range(KCH):
                    nc.tensor.matmul(psg, lhsT=xTh[k][:, ts(t, P)],
                                     rhs=Wgh_sb[:, k, :],
                                     start=(k == 0), stop=False)
                    nc.tensor.matmul(psg, lhsT=xTh[k][:, ts(t, P)],
                                     rhs=Wgl_sb[:, k, :],
                                     start=False, stop=False)
                    nc.tensor.matmul(psg, lhsT=xTl[k][:, ts(t, P)],
                                     rhs=Wgh_sb[:, k, :],
                                     start=False, stop=False)
                nc.tensor.matmul(psg, lhsT=ones_row, rhs=bg_sb,
                                 start=False, stop=True)
                logits = small.tile([P, E], F32, tag="logits")
                nc.vector.tensor_copy(logits, psg)
                maxes = small.tile([P, E], F32, tag="maxes")
                nc.vector.max(maxes, logits)
                negm1 = small.tile([P, 1], F32, tag="negm1")
                nc.vector.tensor_scalar_mul(negm1, maxes[:, 0:1], -1.0)
                p = small.tile([P, E], F32, tag="p")
                nc.scalar.activation(p, logits,
                                     mybir.ActivationFunctionType.Exp,
                                     bias=negm1, scale=1.0)
                # top-2 of p (p1 = 1.0 at argmax); exact values for matching
                pmax = small.tile([P, E], F32, tag="pmax")
                nc.vector.max(pmax, p)
                repl = small.tile([P, E], F32, tag="repl")
                nc.vector.memset(repl, -1.0)  # p > 0, never matches
                nc.vector.tensor_copy(repl[:, 0:2], pmax[:, 0:2])
                denom = small.tile([P, 1], F32, tag="denom")
                nc.vector.tensor_add(denom, pmax[:, 0:1], pmax[:, 1:2])
                rec = small.tile([P, 1], F32, tag="rec")
                nc.vector.reciprocal(rec, denom)
                pm = small.tile([P, E], F32, tag="pm")
                nc.vector.match_replace(out=pm, in_to_replace=repl,
                                        in_values=p, imm_value=0.0)
                nc.vector.tensor_sub(pm, p, pm)  # top-2 values, else 0
                nc.vector.tensor_scalar_mul(w_all[:, ds(t * E, E)], pm, rec)
                pst = psum_sm.tile([E, P], F32, tag="pst")
                nc.tensor.transpose(pst, w_all[:, ds(t * E, E)], ident)
                nc.vector.tensor_copy(wT_sb[:, ts(t, P)], pst)

            # ---- bias init: acc[t] = w_t^T @ be ----
            acc = []
            for t in range(TCH):
                acc_t = acc_pool.tile([P, O], F32, tag="acc")
                for h in range(OH):
                    psb = psum_mm.tile([P, 512], F32, tag="mm")
                    nc.tensor.matmul(psb, lhsT=wT_sb[:, ts(t, P)],
                                     rhs=be_sb[:, ds(h * 512, 512)],
                                     start=True, stop=True)
                    nc.scalar.activation(acc_t[:, ds(h * 512, 512)], psb,
                                         mybir.ActivationFunctionType.Copy)
                acc.append(acc_t)

            # ---- main: per-expert dense matmul + fused scale-accumulate ----
            for e in range(E):
                if e + 2 < E:
                    load_expert(e + 2)
                wt = wt_all.pop(e)
                for h in range(OH):
                    for t in range(TCH):
                        ps = psum_mm.tile([P, 512], F32, tag="mm")
                        for k in range(KCH):
                            nc.tensor.matmul(
                                ps,
                                lhsT=xT[k][:, ts(t, P)],
                                rhs=wt[k][:, ds(h * 512, 512)],
                                start=(k == 0), stop=(k == KCH - 1))
                        nc.vector.scalar_tensor_tensor(
                            out=acc[t][:, ds(h * 512, 512)],
                            in0=ps,
                            scalar=w_all[:, ds(t * E + e, 1)],
                            in1=acc[t][:, ds(h * 512, 512)],
                            op0=mybir.AluOpType.mult,
                            op1=mybir.AluOpType.add)

            # ---- store ----
            for t in range(TCH):
                nc.sync.dma_start(out=out_d[ts(t, P), :], in_=acc[t])

    nc.compile()
    return nc


_NC_CACHE = None
last_results = None  # BassKernelResults from the most recent run (for test.py)


def _get_nc():
    global _NC_CACHE
    if _NC_CACHE is None:
        _NC_CACHE = _build()
    return _NC_CACHE


def _hi_lo(a):
    import ml_dtypes
    hi = a.astype(ml_dtypes.bfloat16)
    lo = (a - hi.astype(np.float32)).astype(ml_dtypes.bfloat16)
    return hi, lo


def kernel(x, We, be, Wg, bg):
    global last_results
    x = np.ascontiguousarray(np.asarray(x, dtype=np.float32))
    We_np = np.ascontiguousarray(np.asarray(We, dtype=np.float32))
    be_np = np.ascontiguousarray(np.asarray(be, dtype=np.float32))
    Wg_np = np.ascontiguousarray(np.asarray(Wg, dtype=np.float32))
    bg_np = np.ascontiguousarray(np.asarray(bg, dtype=np.float32)).reshape(1, E)
    Wgh, Wgl = _hi_lo(Wg_np)

    x_flat = x.reshape(N, D)
    in_maps = []
    for c in range(NCORES):
        xT_c = np.ascontiguousarray(x_flat[c * NT:(c + 1) * NT].T)
        xTh_c, xTl_c = _hi_lo(xT_c)
        in_maps.append({"xT": xT_c, "We": We_np, "be": be_np,
                        "xTh": xTh_c, "xTl": xTl_c,
                        "Wgh": Wgh, "Wgl": Wgl, "bg": bg_np})

    last_results = run_bass_kernel_spmd(_get_nc(), in_maps,
                                        core_ids=list(range(NCORES)))
    out = np.concatenate([r["out"] for r in last_results.results], axis=0)
    return out.reshape(B, S, O)
